# revision 5
# baseline (speedup 1.0000x reference)
"""Mamba2 mixer kernel for 8 trn2 NeuronCores, tensor-parallel over heads.

Each core k handles heads 8k..8k+7 (d_inner channels 512k..512k+512):
  - in_proj slice (z, x, dt columns; B/C computed redundantly on all cores)
  - causal depthwise conv + silu over its x channels + B/C
  - chunked SSD scan for its 8 heads (chunk size 256)
  - gated output y * silu(z); RMSNorm over the full 4096 channels uses an
    AllReduce of per-token partial sums of squares.
Host only reshapes/slices inputs and concatenates the 8 output slices.
"""

import numpy as np

import concourse.bass as bass
import concourse.tile as tile
from concourse import bacc, mybir
from concourse.bass_utils import run_bass_kernel_spmd

F32 = mybir.dt.float32
F32R = mybir.dt.float32r
AF = mybir.ActivationFunctionType
ALU = mybir.AluOpType

# dims
B_, L_, DM = 2, 2048, 2048
DS, DC, HD, NG = 128, 4, 64, 1
DI = 2 * DM              # 4096
NH = DI // HD            # 64
NCORE = 8
HPC = NH // NCORE        # 8 heads per core
XC = DI // NCORE         # 512 x/z channels per core
T = B_ * L_              # 4096 tokens
SEG = 256                # tokens per segment == one SSD chunk
NSEG = T // SEG          # 16
SPB = NSEG // B_         # segments per batch
NKB = DM // 128          # 16 contraction blocks
NBLK = T // 128          # 32 token blocks
EPS = 1e-5
NEG = -1e30


def _build_nc():
    nc = bacc.Bacc("TRN2", target_bir_lowering=False, num_devices=NCORE)

    uT = nc.dram_tensor("uT", [DM, T], F32R, kind="ExternalInput")
    w_xbc = nc.dram_tensor("w_xbc", [DM, 776], F32R, kind="ExternalInput")
    w_z = nc.dram_tensor("w_z", [DM, XC], F32R, kind="ExternalInput")
    convw = nc.dram_tensor("convw", [128, 24], F32, kind="ExternalInput")
    convb = nc.dram_tensor("convb", [128, 6], F32, kind="ExternalInput")
    dtb = nc.dram_tensor("dtb", [HPC, 1], F32, kind="ExternalInput")
    alog = nc.dram_tensor("alog", [HPC, 1], F32, kind="ExternalInput")
    dsk = nc.dram_tensor("dsk", [HPC, 1], F32, kind="ExternalInput")
    nrmw = nc.dram_tensor("nrmw", [HPC, XC], F32, kind="ExternalInput")
    onehot = nc.dram_tensor("onehot", [HPC, 9 * 128], F32, kind="ExternalInput")
    maskadd = nc.dram_tensor("maskadd", [128, 2 * SEG], F32, kind="ExternalInput")
    ident = nc.dram_tensor("ident", [128, 128], F32, kind="ExternalInput")

    out = nc.dram_tensor("out", [T, XC], F32, kind="ExternalOutput")

    with tile.TileContext(nc) as tc:
        with (
            tc.tile_pool(name="wpool", bufs=1) as wpool,
            tc.tile_pool(name="cpool", bufs=1) as cpool,
            tc.tile_pool(name="upool", bufs=2) as upool,
            tc.tile_pool(name="xpool", bufs=2) as xpool,
            tc.tile_pool(name="ypool", bufs=2) as ypool,
            tc.tile_pool(name="wk2", bufs=2) as wk2,
            tc.tile_pool(name="w512", bufs=3) as w512,
            tc.tile_pool(name="hwork", bufs=2) as hwork,
            tc.tile_pool(name="rpool", bufs=2) as rpool,
            tc.tile_pool(name="ps_big", bufs=2, space="PSUM") as ps_big,
            tc.tile_pool(name="ps_bc", bufs=2, space="PSUM") as ps_bc,
            tc.tile_pool(name="ps_sm", bufs=4, space="PSUM") as ps_sm,
            tc.tile_pool(name="dram", bufs=1, space="DRAM") as dram,
        ):
            # ---------------- constants / weights -------------------------
            wx_sb = wpool.tile([128, NKB, 776], F32R)
            nc.sync.dma_start(wx_sb[:], w_xbc.rearrange("(o p) c -> p o c", p=128))
            wz_sb = wpool.tile([128, NKB, XC], F32R)
            nc.sync.dma_start(wz_sb[:], w_z.rearrange("(o p) c -> p o c", p=128))
            cw_sb = cpool.tile([128, 24], F32)
            nc.sync.dma_start(cw_sb[:], convw[:])
            cb_sb = cpool.tile([128, 6], F32)
            nc.sync.dma_start(cb_sb[:], convb[:])
            dtb_sb = cpool.tile([HPC, 1], F32)
            nc.sync.dma_start(dtb_sb[:], dtb[:])
            alog_sb = cpool.tile([HPC, 1], F32)
            nc.sync.dma_start(alog_sb[:], alog[:])
            dsk_sb = cpool.tile([HPC, 1], F32)
            nc.sync.dma_start(dsk_sb[:], dsk[:])
            oh_sb = cpool.tile([HPC, 9 * 128], F32)
            nc.sync.dma_start(oh_sb[:], onehot[:])
            mk_sb = cpool.tile([128, 2, SEG], F32)
            nc.sync.dma_start(mk_sb[:], maskadd.rearrange("p (b t) -> p b t", b=2))
            id_sb = cpool.tile([128, 128], F32)
            nc.sync.dma_start(id_sb[:], ident[:])
            ones8 = oh_sb[:, 8 * 128 : 9 * 128]          # [8,128] all ones
            nrm8 = cpool.tile([HPC, XC], F32)
            nc.sync.dma_start(nrm8[:], nrmw[:])

            # A = -exp(A_log)
            a_sb = cpool.tile([HPC, 1], F32)
            nc.scalar.activation(a_sb[:], alog_sb[:], AF.Exp)
            nc.vector.tensor_scalar_mul(a_sb[:], a_sb[:], -1.0)

            # broadcast D_skip and norm_w to 128 partitions via ones matmul
            dgD = cpool.tile([HPC, HPC], F32)
            nc.vector.tensor_scalar_mul(dgD[:], id_sb[0:HPC, 0:HPC], dsk_sb[:, 0:1])
            pD = ps_sm.tile([128, 256], F32, tag="sm")
            nc.tensor.matmul(pD[:, 0:HPC], ones8, dgD[:], start=True, stop=True)
            D_bc = cpool.tile([128, HPC], F32)
            nc.vector.tensor_copy(D_bc[:], pD[:, 0:HPC])

            nrm_bc = cpool.tile([128, XC], F32)
            for q in range(2):
                pN = ps_sm.tile([128, 256], F32, tag="sm")
                nc.tensor.matmul(
                    pN[:],
                    oh_sb[:, 0:128],
                    nrm8[:, q * 256 : (q + 1) * 256],
                    start=True,
                    stop=True,
                )
                nc.vector.tensor_copy(nrm_bc[:, q * 256 : (q + 1) * 256], pN[:])

            # DRAM scratch
            yg_dram = dram.tile([T // 128, 128, XC], F32)
            cc_in = dram.tile([128, NBLK], F32)
            cc_out = dram.tile([128, NBLK], F32)

            partial = cpool.tile([128, NBLK], F32)

            # persistent SSD carry state per head
            R_cur = [None] * HPC
            prev_xsrc = None

            # ---------------- main loop over segments(=chunks) ------------
            for seg in range(NSEG):
                first = seg % SPB == 0
                # u^T tiles for this segment
                ut = upool.tile([128, NKB, SEG], F32R, tag="ut")
                nc.sync.dma_start(
                    ut[:],
                    uT.rearrange("(o p) t -> p o t", p=128)[
                        :, :, seg * SEG : (seg + 1) * SEG
                    ],
                )

                # ---- in_proj orientation 1: [ch, t] for x/B/C/dt --------
                xsrc = xpool.tile([128, 6, SEG + 3], F32, tag="xsrc")
                for cb in range(6):
                    p1 = ps_big.tile([128, XC], F32, tag="big")
                    for kb in range(NKB):
                        nc.tensor.matmul(
                            p1[:, 0:SEG],
                            wx_sb[:, kb, cb * 128 : (cb + 1) * 128],
                            ut[:, kb, :],
                            start=(kb == 0),
                            stop=(kb == NKB - 1),
                        )
                    nc.vector.tensor_copy(xsrc[:, cb, 3 : SEG + 3], p1[:, 0:SEG])
                # dt columns (8 wide)
                pdt = ps_sm.tile([128, 256], F32, tag="sm")
                for kb in range(NKB):
                    nc.tensor.matmul(
                        pdt[0:HPC, :],
                        wx_sb[:, kb, 768:776],
                        ut[:, kb, :],
                        start=(kb == 0),
                        stop=(kb == NKB - 1),
                    )
                dtraw = wk2.tile([HPC, SEG], F32, tag="dtraw")
                nc.vector.tensor_copy(dtraw[:], pdt[0:HPC, :])

                # conv halo
                if first:
                    nc.vector.memset(xsrc[:, :, 0:3], 0.0)
                else:
                    nc.vector.tensor_copy(
                        xsrc[:, :, 0:3], prev_xsrc[:, :, SEG : SEG + 3]
                    )
                prev_xsrc = xsrc

                # ---- in_proj orientation 2: z [t, ch] + silu ------------
                zs = ypool.tile([128, 2, XC], F32, tag="zsil")
                for tb in range(2):
                    pz = ps_big.tile([128, XC], F32, tag="big")
                    for kb in range(NKB):
                        nc.tensor.matmul(
                            pz[:],
                            ut[:, kb, tb * 128 : (tb + 1) * 128],
                            wz_sb[:, kb, :],
                            start=(kb == 0),
                            stop=(kb == NKB - 1),
                        )
                    ez = w512.tile([128, XC], F32, tag="w512")
                    nc.scalar.activation(ez[:], pz[:], AF.Exp, scale=-1.0)
                    nc.vector.tensor_scalar_add(ez[:], ez[:], 1.0)
                    nc.vector.reciprocal(ez[:], ez[:])
                    nc.vector.tensor_mul(zs[:, tb, :], pz[:], ez[:])

                # ---- conv + silu ----------------------------------------
                xc = xpool.tile([128, 4, SEG], F32, tag="xconv")   # x channels
                bc = xpool.tile([128, 2, SEG], F32, tag="bcconv")  # B, C
                for cb in range(6):
                    acc = wk2.tile([128, SEG], F32, tag="acc")
                    nc.vector.tensor_scalar(
                        acc[:],
                        xsrc[:, cb, 0:SEG],
                        cw_sb[:, 4 * cb : 4 * cb + 1],
                        cb_sb[:, cb : cb + 1],
                        ALU.mult,
                        ALU.add,
                    )
                    for k in range(1, 4):
                        nc.vector.scalar_tensor_tensor(
                            acc[:],
                            xsrc[:, cb, k : k + SEG],
                            cw_sb[:, 4 * cb + k : 4 * cb + k + 1],
                            acc[:],
                            ALU.mult,
                            ALU.add,
                        )
                    ec = wk2.tile([128, SEG], F32, tag="ec")
                    nc.scalar.activation(ec[:], acc[:], AF.Exp, scale=-1.0)
                    nc.vector.tensor_scalar_add(ec[:], ec[:], 1.0)
                    nc.vector.reciprocal(ec[:], ec[:])
                    dst = xc[:, cb, :] if cb < 4 else bc[:, cb - 4, :]
                    nc.vector.tensor_mul(dst, acc[:], ec[:])

                # ---- dt -> softplus -> dA -> Acs ------------------------
                dsp = wk2.tile([HPC, SEG], F32, tag="dsp")
                nc.scalar.activation(dsp[:], dtraw[:], AF.Exp, bias=dtb_sb[:, 0:1])
                nc.vector.tensor_scalar_add(dsp[:], dsp[:], 1.0)
                nc.scalar.activation(dsp[:], dsp[:], AF.Ln)
                dA = wk2.tile([HPC, SEG], F32, tag="dA")
                nc.vector.tensor_scalar_mul(dA[:], dsp[:], a_sb[:, 0:1])
                acs = wk2.tile([HPC, SEG], F32, tag="acs")
                nc.vector.tensor_tensor_scan(
                    acs[:], dA[:], dA[:], 0.0, ALU.add, ALU.bypass
                )

                # transposes of dt_softplus and Acs -> [t, h]; exp(Acs) cols
                dtT = wk2.tile([128, 2, HPC], F32, tag="dtT")
                acsT = wk2.tile([128, 2, HPC], F32, tag="acsT")
                eaT = wk2.tile([128, 2, HPC], F32, tag="eaT")
                for tb in range(2):
                    pt = ps_sm.tile([128, 256], F32, tag="sm")
                    nc.tensor.transpose(
                        pt[:, 0:HPC],
                        dsp[:, tb * 128 : (tb + 1) * 128],
                        id_sb[0:HPC, 0:HPC],
                    )
                    nc.vector.tensor_copy(dtT[:, tb, :], pt[:, 0:HPC])
                    pa = ps_sm.tile([128, 256], F32, tag="sm")
                    nc.tensor.transpose(
                        pa[:, 0:HPC],
                        acs[:, tb * 128 : (tb + 1) * 128],
                        id_sb[0:HPC, 0:HPC],
                    )
                    nc.vector.tensor_copy(acsT[:, tb, :], pa[:, 0:HPC])
                    nc.scalar.activation(eaT[:, tb, :], acsT[:, tb, :], AF.Exp)

                # exp(Alast) broadcast to 128 partitions: [128, 8]
                eal8 = wk2.tile([HPC, 1], F32, tag="eal8")
                nc.scalar.activation(eal8[:], acs[:, SEG - 1 : SEG], AF.Exp)
                dg = wk2.tile([HPC, HPC], F32, tag="dg")
                nc.vector.tensor_scalar_mul(dg[:], id_sb[0:HPC, 0:HPC], eal8[:, 0:1])
                pe = ps_sm.tile([128, 256], F32, tag="sm")
                nc.tensor.matmul(pe[:, 0:HPC], ones8, dg[:], start=True, stop=True)
                eal_bc = wk2.tile([128, HPC], F32, tag="ealbc")
                nc.vector.tensor_copy(eal_bc[:], pe[:, 0:HPC])

                # ---- B transpose, BC^T matmul ---------------------------
                BT = wk2.tile([128, 2, 128], F32, tag="BT")
                for tb in range(2):
                    pb = ps_sm.tile([128, 256], F32, tag="sm")
                    nc.tensor.transpose(
                        pb[:, 0:128],
                        bc[:, 0, tb * 128 : (tb + 1) * 128],
                        id_sb[:],
                    )
                    nc.vector.tensor_copy(BT[:, tb, :], pb[:, 0:128])

                pbc = []
                for sb_ in range(2):
                    pc = ps_bc.tile([128, SEG], F32, tag="bc")
                    nc.tensor.matmul(
                        pc[:],
                        bc[:, 0, sb_ * 128 : (sb_ + 1) * 128],
                        bc[:, 1, :],
                        start=True,
                        stop=True,
                    )
                    pbc.append(pc)

                # ---- x transposes: [s, ch] ------------------------------
                xT = xpool.tile([128, 2, XC], F32, tag="xT")
                for xb in range(4):
                    for tb in range(2):
                        px = ps_sm.tile([128, 256], F32, tag="sm")
                        nc.tensor.transpose(
                            px[:, 0:128],
                            xc[:, xb, tb * 128 : (tb + 1) * 128],
                            id_sb[:],
                        )
                        nc.vector.tensor_copy(
                            xT[:, tb, xb * 128 : (xb + 1) * 128], px[:, 0:128]
                        )

                # ---- per-head SSD ---------------------------------------
                y_sb = ypool.tile([128, 2, XC], F32, tag="y")
                for h in range(HPC):
                    # broadcast Acs row h
                    pab = ps_sm.tile([128, 256], F32, tag="sm")
                    nc.tensor.matmul(
                        pab[:],
                        oh_sb[:, h * 128 : (h + 1) * 128],
                        acs[:],
                        start=True,
                        stop=True,
                    )
                    est = hwork.tile([128, 2, SEG], F32, tag="est")
                    for sb_ in range(2):
                        nc.vector.scalar_tensor_tensor(
                            est[:, sb_, :],
                            pab[:],
                            acsT[:, sb_, h : h + 1],
                            mk_sb[:, sb_, :],
                            ALU.subtract,
                            ALU.add,
                        )
                        nc.scalar.activation(est[:, sb_, :], est[:, sb_, :], AF.Exp)
                    # M[s,t] = est * dt[s] * BC[s,t]
                    M = hwork.tile([128, 2, SEG], F32, tag="M")
                    for sb_ in range(2):
                        nc.vector.scalar_tensor_tensor(
                            M[:, sb_, :],
                            est[:, sb_, :],
                            dtT[:, sb_, h : h + 1],
                            pbc[sb_][:],
                            ALU.mult,
                            ALU.mult,
                        )
                    # Yd
                    pys = []
                    for tb in range(2):
                        py = ps_sm.tile([128, 256], F32, tag="sm")
                        for sb_ in range(2):
                            nc.tensor.matmul(
                                py[:, 0:HD],
                                M[:, sb_, tb * 128 : (tb + 1) * 128],
                                xT[:, sb_, h * HD : (h + 1) * HD],
                                start=(sb_ == 0),
                                stop=(sb_ == 1),
                            )
                        pys.append(py)
                    # scaled B for states
                    Bs = hwork.tile([128, 2, 128], F32, tag="Bs")
                    for sb_ in range(2):
                        nc.vector.tensor_scalar(
                            Bs[:, sb_, :],
                            BT[:, sb_, :],
                            est[:, sb_, SEG - 1 : SEG],
                            dtT[:, sb_, h : h + 1],
                            ALU.mult,
                            ALU.mult,
                        )
                    ps_st = ps_sm.tile([128, 256], F32, tag="sm")
                    for sb_ in range(2):
                        nc.tensor.matmul(
                            ps_st[:, 0:HD],
                            Bs[:, sb_, :],
                            xT[:, sb_, h * HD : (h + 1) * HD],
                            start=(sb_ == 0),
                            stop=(sb_ == 1),
                        )
                    # Yo from carried state (before R update)
                    pyo = []
                    if not first:
                        for tb in range(2):
                            po = ps_sm.tile([128, 256], F32, tag="sm")
                            nc.tensor.matmul(
                                po[:, 0:HD],
                                bc[:, 1, tb * 128 : (tb + 1) * 128],
                                R_cur[h][:],
                                start=True,
                                stop=True,
                            )
                            pyo.append(po)
                    # y = x*D + Yd (+ Yo * expAcs)   [before R update frees ps_st]
                    for tb in range(2):
                        ysl = y_sb[:, tb, h * HD : (h + 1) * HD]
                        nc.vector.scalar_tensor_tensor(
                            ysl,
                            xT[:, tb, h * HD : (h + 1) * HD],
                            D_bc[:, h : h + 1],
                            pys[tb][:, 0:HD],
                            ALU.mult,
                            ALU.add,
                        )
                        if not first:
                            nc.vector.scalar_tensor_tensor(
                                ysl,
                                pyo[tb][:, 0:HD],
                                eaT[:, tb, h : h + 1],
                                ysl,
                                ALU.mult,
                                ALU.add,
                            )
                    # R update
                    Rn = rpool.tile([128, HD], F32, tag=f"R{h}")
                    if first:
                        nc.vector.tensor_copy(Rn[:], ps_st[:, 0:HD])
                    else:
                        nc.vector.scalar_tensor_tensor(
                            Rn[:],
                            R_cur[h][:],
                            eal_bc[:, h : h + 1],
                            ps_st[:, 0:HD],
                            ALU.mult,
                            ALU.add,
                        )
                    R_cur[h] = Rn

                # ---- gate + partial sumsq + spill yg --------------------
                for tb in range(2):
                    yg = w512.tile([128, XC], F32, tag="w512")
                    nc.vector.tensor_mul(yg[:], y_sb[:, tb, :], zs[:, tb, :])
                    sq = w512.tile([128, XC], F32, tag="w512")
                    nc.scalar.activation(
                        sq[:],
                        yg[:],
                        AF.Square,
                        accum_out=partial[:, seg * 2 + tb : seg * 2 + tb + 1],
                    )
                    nc.sync.dma_start(yg_dram[seg * 2 + tb], yg[:])

            # ---------------- allreduce + normalize -----------------------
            nc.sync.dma_start(cc_in[:], partial[:])
            nc.gpsimd.collective_compute(
                "AllReduce",
                ALU.add,
                replica_groups=[list(range(NCORE))],
                ins=[cc_in.opt()],
                outs=[cc_out.opt()],
            )
            tot = cpool.tile([128, NBLK], F32)
            nc.sync.dma_start(tot[:], cc_out[:])
            epsc = cpool.tile([128, 1], F32)
            nc.vector.memset(epsc[:], EPS)
            scl = cpool.tile([128, NBLK], F32)
            nc.scalar.activation(scl[:], tot[:], AF.Ln, bias=epsc[:], scale=1.0 / DI)
            nc.scalar.activation(scl[:], scl[:], AF.Exp, scale=-0.5)

            for blk in range(T // 128):
                ygr = w512.tile([128, XC], F32, tag="w512")
                nc.sync.dma_start(ygr[:], yg_dram[blk])
                nc.vector.tensor_scalar_mul(ygr[:], ygr[:], scl[:, blk : blk + 1])
                nc.vector.tensor_mul(ygr[:], ygr[:], nrm_bc[:])
                nc.sync.dma_start(out[blk * 128 : (blk + 1) * 128, :], ygr[:])

    nc.compile()
    return nc


_NC = None


def _host_inputs(u, w_in, conv_w, conv_b, dt_bias, A_log, D_skip, norm_w):
    """Build the 8 per-core input dicts."""
    u2 = np.ascontiguousarray(u.reshape(T, DM).T)          # [DM, T]
    onehot = np.zeros((HPC, 9 * 128), np.float32)
    for h in range(HPC):
        onehot[h, h * 128 : (h + 1) * 128] = 1.0
    onehot[:, 8 * 128 : 9 * 128] = 1.0                     # all-ones block
    mask = np.zeros((128, 2 * SEG), np.float32)
    for sb_ in range(2):
        for p in range(128):
            s = sb_ * 128 + p
            mask[p, sb_ * SEG : sb_ * SEG + s] = NEG       # t < s masked
    ident = np.eye(128, dtype=np.float32)

    ins = []
    for k in range(NCORE):
        xcols = np.arange(DI + k * XC, DI + (k + 1) * XC)
        bcols = np.arange(2 * DI, 2 * DI + 2 * DS)
        dtcols = np.arange(
            2 * DI + 2 * DS + k * HPC, 2 * DI + 2 * DS + (k + 1) * HPC
        )
        w_xbc = np.ascontiguousarray(
            np.concatenate([w_in[:, xcols], w_in[:, bcols], w_in[:, dtcols]], 1)
        )                                                   # [DM, 776]
        w_z = np.ascontiguousarray(w_in[:, k * XC : (k + 1) * XC])
        chans = np.concatenate(
            [np.arange(k * XC, (k + 1) * XC), np.arange(DI, DI + 2 * DS)]
        )
        cw = conv_w[chans]                                  # [768, 4]
        cb = conv_b[chans]
        cw_p = np.zeros((128, 24), np.float32)
        cb_p = np.zeros((128, 6), np.float32)
        for cbk in range(6):
            cw_p[:, 4 * cbk : 4 * cbk + 4] = cw[cbk * 128 : (cbk + 1) * 128]
            cb_p[:, cbk] = cb[cbk * 128 : (cbk + 1) * 128]
        ins.append(
            dict(
                uT=u2,
                w_xbc=w_xbc,
                w_z=w_z,
                convw=cw_p,
                convb=cb_p,
                dtb=np.ascontiguousarray(dt_bias[k * HPC : (k + 1) * HPC, None]),
                alog=np.ascontiguousarray(A_log[k * HPC : (k + 1) * HPC, None]),
                dsk=np.ascontiguousarray(D_skip[k * HPC : (k + 1) * HPC, None]),
                nrmw=np.ascontiguousarray(
                    np.repeat(norm_w[None, k * XC : (k + 1) * XC], HPC, 0)
                ),
                onehot=onehot,
                maskadd=mask,
                ident=ident,
            )
        )
    return ins


def kernel(u, w_in, conv_w, conv_b, dt_bias, A_log, D_skip, norm_w):
    global _NC
    u = np.asarray(u, np.float32)
    w_in = np.asarray(w_in, np.float32)
    conv_w = np.asarray(conv_w, np.float32)
    conv_b = np.asarray(conv_b, np.float32)
    dt_bias = np.asarray(dt_bias, np.float32)
    A_log = np.asarray(A_log, np.float32)
    D_skip = np.asarray(D_skip, np.float32)
    norm_w = np.asarray(norm_w, np.float32)

    if _NC is None:
        _NC = _build_nc()
    ins = _host_inputs(u, w_in, conv_w, conv_b, dt_bias, A_log, D_skip, norm_w)
    res = run_bass_kernel_spmd(_NC, ins, core_ids=list(range(NCORE)))
    full = np.concatenate([res.results[k]["out"] for k in range(NCORE)], axis=1)
    return full.reshape(B_, L_, DI)
